# revision 8
# baseline (speedup 1.0000x reference)
"""BiLSTM-CRF kernel for 8 TRN2 NeuronCores.

Device (8 cores): the two LSTM input-projection GEMMs, sharded
direction-parallel x time-parallel — cores 0-3 compute the forward
projection for one quarter of the timesteps each, cores 4-7 the backward
projection.
Host: embedding gather, the two sequential LSTM scans, output projection,
Viterbi decode.
"""

import sys

import numpy as np

for p in ("/opt/trn_rl_repo", "/opt/pypackages"):
    if p not in sys.path:
        sys.path.append(p)

import concourse.bass as bass
import concourse.mybir as mybir
from concourse import bacc
from concourse.bass_utils import run_bass_kernel_spmd
from concourse.tile import TileContext

T, V, E, H = 4096, 50257, 1024, 1024
HH = H // 2
K, START, STOP = 16, 14, 15
NEG = -10000.0

NCORES = 8
NDP = 4                    # time-parallel ways (per direction)
TSH = T // NDP             # 1024 timesteps per core
P = 128
KC = E // P                # 8 contraction chunks
GH = 4 * HH                # 2048 gate rows (one direction per core group)
MC = GH // P               # 16 output-row chunks
NT = 2                     # psum n-chunks (fp32 moving max 512)
NS = TSH // NT

_CACHED = {}


def _build_graph():
    nc = bacc.Bacc(None)
    f32 = mybir.dt.float32

    emb_t = nc.declare_dram_parameter("emb_t", [E, TSH], f32, isOutput=False)
    w_t = nc.declare_dram_parameter("w_t", [E, GH], f32, isOutput=False)
    out_ext = nc.declare_dram_parameter("out", [GH, TSH], f32, isOutput=True)

    emb_r = emb_t.rearrange("(kc p) t -> p kc t", p=P)
    w_r = w_t.rearrange("(kc p) m -> p kc m", p=P)

    with TileContext(nc) as tc:
        with (
            tc.tile_pool(name="embp", bufs=1) as embp,
            tc.tile_pool(name="wp", bufs=1) as wp,
            tc.tile_pool(name="op", bufs=3) as op,
            tc.tile_pool(name="ps", bufs=4, space="PSUM") as psp,
        ):
            emb_tile = embp.tile([P, KC, TSH], f32)
            for k in range(KC):
                nc.sync.dma_start(out=emb_tile[:, k], in_=emb_r[:, k])
            wt_all = wp.tile([P, KC, GH], f32)  # 64KB/partition, loaded once
            for k in range(KC):
                nc.sync.dma_start(out=wt_all[:, k], in_=w_r[:, k])
            for m in range(MC):
                ot = op.tile([P, TSH], f32)
                for n in range(NT):
                    ps = psp.tile([P, NS], f32)
                    for k in range(KC):
                        nc.tensor.matmul(
                            ps, wt_all[:, k, bass.ts(m, P)],
                            emb_tile[:, k, bass.ts(n, NS)],
                            start=(k == 0), stop=(k == KC - 1),
                        )
                    nc.vector.tensor_copy(ot[:, bass.ts(n, NS)], ps)
                nc.gpsimd.dma_start(out=out_ext[bass.ts(m, P), :], in_=ot)
    nc.finalize()
    return nc


def _device_projections(emb, w_ih_f, w_ih_b, trace=False):
    """pre_f, pre_b = emb @ w_ih_f.T, emb @ w_ih_b.T via 8-core SPMD."""
    if "nc" not in _CACHED:
        _CACHED["nc"] = _build_graph()
    nc = _CACHED["nc"]

    w_f_t = np.ascontiguousarray(w_ih_f.T.astype(np.float32))  # [E, 2048]
    w_b_t = np.ascontiguousarray(w_ih_b.T.astype(np.float32))
    emb_sh = [
        np.ascontiguousarray(emb[d * TSH:(d + 1) * TSH].T.astype(np.float32))
        for d in range(NDP)
    ]
    in_maps = []
    for i in range(NCORES):
        g, d = divmod(i, NDP)  # cores 0-3 forward, 4-7 backward
        in_maps.append({"emb_t": emb_sh[d], "w_t": w_f_t if g == 0 else w_b_t})

    res = run_bass_kernel_spmd(nc, in_maps, list(range(NCORES)), trace=trace)
    outs = [res.results[i]["out"] for i in range(NCORES)]  # each [GH, TSH]
    pre_f = np.concatenate(outs[:NDP], axis=1).T   # [T, 2048]
    pre_b = np.concatenate(outs[NDP:], axis=1).T
    return pre_f, pre_b, res


def _sigmoid(x):
    out = np.empty_like(x)
    pos = x >= 0
    out[pos] = 1.0 / (1.0 + np.exp(-x[pos], dtype=np.float32))
    ex = np.exp(x[~pos], dtype=np.float32)
    out[~pos] = ex / (1.0 + ex)
    return out


def _scan_dir(pre, w_hh, b_hh, h0, c0):
    Tn = pre.shape[0]
    w_hh_t = np.ascontiguousarray(w_hh.T.astype(np.float32))
    h = h0.astype(np.float32).copy()
    c = c0.astype(np.float32).copy()
    hs = np.empty((Tn, HH), np.float32)
    bias = b_hh.astype(np.float32)
    for t in range(Tn):
        g = pre[t] + h @ w_hh_t + bias
        i = _sigmoid(g[:HH])
        f = _sigmoid(g[HH:2 * HH])
        gg = np.tanh(g[2 * HH:3 * HH])
        o = _sigmoid(g[3 * HH:])
        c = f * c + i * gg
        h = o * np.tanh(c)
        hs[t] = h
    return hs


def _viterbi(feats, transitions):
    prev = np.full((K,), NEG, np.float32)
    prev[START] = 0.0
    bptrs = np.empty((T, K), np.int64)
    for t in range(T):
        scores = prev[None, :] + transitions  # [next, prev]
        bptrs[t] = np.argmax(scores, axis=1)
        prev = scores.max(axis=1) + feats[t]
    terminal = prev + transitions[STOP]
    best = int(np.argmax(terminal))
    path_score = terminal[best]
    best_path = np.empty((T,), np.int32)
    tag = best
    for t in range(T - 1, -1, -1):
        best_path[t] = tag
        tag = int(bptrs[t, tag])
    return np.float32(path_score), best_path


def kernel(sentence, embed, w_ih_f, w_hh_f, b_ih_f, b_hh_f,
           w_ih_b, w_hh_b, b_ih_b, b_hh_b, h0, c0, w_out, b_out,
           transitions, _trace=False):
    sentence = np.asarray(sentence)
    embed = np.asarray(embed, np.float32)
    emb = embed[sentence]  # [T, E]

    pre_f, pre_b_t, res = _device_projections(
        emb, np.asarray(w_ih_f, np.float32), np.asarray(w_ih_b, np.float32),
        trace=_trace,
    )
    _CACHED["last_res"] = res

    pre_f = pre_f + np.asarray(b_ih_f, np.float32)
    # backward scan runs over reversed time
    pre_b = pre_b_t[::-1] + np.asarray(b_ih_b, np.float32)

    h0 = np.asarray(h0, np.float32)
    c0 = np.asarray(c0, np.float32)
    hf = _scan_dir(pre_f, np.asarray(w_hh_f, np.float32),
                   np.asarray(b_hh_f, np.float32), h0[0], c0[0])
    hb = _scan_dir(pre_b, np.asarray(w_hh_b, np.float32),
                   np.asarray(b_hh_b, np.float32), h0[1], c0[1])[::-1]

    lstm_out = np.concatenate([hf, hb], axis=1)  # [T, H]
    feats = lstm_out @ np.asarray(w_out, np.float32).T + np.asarray(b_out, np.float32)
    return _viterbi(feats, np.asarray(transitions, np.float32))
